# revision 22
# baseline (speedup 1.0000x reference)
"""AttractorGuidedNeuroFocalAttention — Trainium2 Bass kernel (8 cores).

Math: reference logits = qk/sqrt(64) - omega, where
  omega = (q2_i + k2_j - 2*qk_ij) * g_i,   g_i = sigmoid(nf_i)
  => logits_ij = (1/8 + 2*g_i)*qk_ij - g_i*k2_j - g_i*q2_i
The "- g_i*q2_i" term is constant along the softmax axis (j) and cancels.
So with c_i = 1/8 + 2*g_i the whole pre-softmax computation folds into one
65-dim augmented matmul:  logits'_ij = [c_i*Q_i, g_i] . [K_j, -k2_j].

Sharding: 16 (batch, head) pairs over 8 cores -> 2 heads per core.
Each core computes scores transposed (partition = keys) so softmax
normalization comes out of the PV matmul for free via an all-ones column
prepended to V (row 0 of the PV output = sum_j exp(logits)_ij).  The final
out-projection partials [2048, 512] are summed on the host per batch.
"""

import sys

for _p in ("/opt/trn_rl_repo", "/root/.axon_site/_ro/trn_rl_repo"):
    if _p not in sys.path:
        sys.path.insert(0, _p)

import numpy as np

import concourse.bacc as bacc
import concourse.bass as bass
import concourse.tile as tile
from concourse import mybir
from concourse.bass_utils import run_bass_kernel_spmd

F32 = mybir.dt.float32
F32R = mybir.dt.float32r
BF16 = mybir.dt.bfloat16
AF = mybir.ActivationFunctionType
ALU = mybir.AluOpType

N = 2048      # sequence length
DIM = 512     # model dim
DH = 64       # head dim
NT = 16       # 128-token tiles
NQS = 4       # 512-query slices (projections)
NQG = 2       # 1024-query groups (attention inner loop)
QW = 1024     # attention query-group width
KC = 16       # 128-key chunks
EPS = 1e-5


def build_nc(niter=1):
    nc = bacc.Bacc(None, target_bir_lowering=False)
    _build_body(nc, niter)
    nc.finalize()
    return nc


def _build_body(nc, niter=1):
    zT = nc.declare_dram_parameter("zT", [DIM, N], F32R, isOutput=False)
    wq = nc.declare_dram_parameter("wq", [DIM, 128], F32R, isOutput=False)
    wk = nc.declare_dram_parameter("wk", [DIM, 128], F32R, isOutput=False)
    wv = nc.declare_dram_parameter("wv", [DIM, 128], F32R, isOutput=False)
    bq2 = nc.declare_dram_parameter("bq2", [64, 2], F32, isOutput=False)
    bk2 = nc.declare_dram_parameter("bk2", [64, 2], F32, isOutput=False)
    bqr = nc.declare_dram_parameter("bqr", [1, 128], F32, isOutput=False)
    woaug = nc.declare_dram_parameter("woaug", [2, 65, DIM], F32R, isOutput=False)
    out = nc.declare_dram_parameter("out", [N, DIM], F32, isOutput=True)

    with tile.TileContext(nc) as tc:
        with (
            tc.tile_pool(name="const", bufs=1) as const,
            tc.tile_pool(name="small", bufs=2) as small,
            tc.tile_pool(name="atp", bufs=2) as atp,
            tc.tile_pool(name="cbp", bufs=2) as cbp,
            tc.tile_pool(name="osbp", bufs=2) as osbp,
            tc.tile_pool(name="dramp", bufs=1, space="DRAM") as dramp,
            tc.tile_pool(name="pspool", bufs=2, space="PSUM") as pspool,
        ):
            # ---------------- Phase A: loads + casts + constants ----------
            zt = const.tile([128, 4, N], F32R, tag="zt")
            zt_bf = const.tile([128, 4, N], BF16, tag="zt_bf")
            zTv = zT.rearrange("(c p) n -> p c n", p=128)
            nc.sync.dma_start(out=zt[:, 0:2, :], in_=zTv[:, 0:2, :])
            nc.sync.dma_start(out=zt[:, 2:4, :], in_=zTv[:, 2:4, :])
            for c in range(4):
                nc.gpsimd.tensor_copy(zt_bf[:, c, :], zt[:, c, :])

            wq_sb = const.tile([128, 4, 128], F32R, tag="wq")
            wk_sb = const.tile([128, 4, 128], F32R, tag="wk")
            wv_sb = const.tile([128, 4, 128], F32R, tag="wv")
            nc.sync.dma_start(out=wq_sb, in_=wq.rearrange("(c p) m -> p c m", p=128))
            nc.sync.dma_start(out=wk_sb, in_=wk.rearrange("(c p) m -> p c m", p=128))
            nc.sync.dma_start(out=wv_sb, in_=wv.rearrange("(c p) m -> p c m", p=128))
            wq_bf = const.tile([128, 4, 128], BF16, tag="wq_bf")
            nc.gpsimd.tensor_copy(wq_bf, wq_sb)

            bq_sb = const.tile([64, 2], F32, tag="bq")
            bk_sb = const.tile([64, 2], F32, tag="bk")
            nc.sync.dma_start(out=bq_sb, in_=bq2[:, :])
            nc.sync.dma_start(out=bk_sb, in_=bk2[:, :])
            bqr_sb = const.tile([1, 128], F32, tag="bqr")
            nc.sync.dma_start(out=bqr_sb, in_=bqr[:, :])
            bqr_bf = const.tile([1, 128], BF16, tag="bqr_bf")
            nc.vector.tensor_copy(bqr_bf, bqr_sb)

            wo_sb = [const.tile([65, DIM], F32R, tag=f"wo{s}", name=f"wo{s}")
                     for s in (0, 1)]
            for s in (0, 1):
                nc.sync.dma_start(out=wo_sb[s], in_=woaug[s, :, :])

            nones64_f = const.tile([64, 1], F32, tag="nones64_f")
            nc.vector.memset(nones64_f, -1.0)
            nones64 = const.tile([64, 1], F32R, tag="nones64")
            nc.vector.tensor_copy(nones64, nones64_f)
            ones_f = const.tile([128, 1], F32, tag="ones_f")
            nc.vector.memset(ones_f, 1.0)
            onesq_bf = const.tile([1, 128], BF16, tag="onesq_bf")
            nc.vector.memset(onesq_bf, 1.0)
            eps_b = const.tile([128, 1], F32, tag="eps_b")
            nc.vector.memset(eps_b, EPS)

            for _it in range(niter):
                qtaug = [const.tile([65, N], F32R, tag=f"qtaug{s}",
                                    name=f"qtaug{s}") for s in (0, 1)]
                ktaug = [const.tile([65, N], F32R, tag=f"ktaug{s}",
                                    name=f"ktaug{s}") for s in (0, 1)]
                otn = [const.tile([65, N], F32R, tag=f"otn{s}",
                                  name=f"otn{s}") for s in (0, 1)]
                grow = [const.tile([1, N], F32R, tag=f"grow{s}",
                                   name=f"grow{s}") for s in (0, 1)]

                # ---- K projection (starts as soon as zt slices land) ------
                for s in (0, 1):
                    for qs in range(NQS):
                        sl = slice(qs * 512, (qs + 1) * 512)
                        kps = pspool.tile([64, 512], F32, tag="mmps", name="kps")
                        for c in range(4):
                            nc.tensor.matmul(
                                kps,
                                lhsT=wk_sb[:, c, s * 64:(s + 1) * 64],
                                rhs=zt[:, c, sl],
                                start=(c == 0),
                                stop=(c == 3),
                            )
                        nc.vector.tensor_scalar_add(ktaug[s][0:64, sl], kps,
                                                    bk_sb[:, s:s + 1])

                # ---- V projection (f32r, ones col prepended) --------------
                v_sb = const.tile([128, 2, NT, 65], F32R, tag="v")
                ones_ap = ones_f[:, :]
                ones_bcast = bass.AP(tensor=ones_ap.tensor, offset=ones_ap.offset,
                                     ap=[list(ones_ap.ap[0]), [0, 2], [0, NT]])
                nc.vector.tensor_copy(v_sb[:, :, :, 0], ones_bcast)
                for t in range(NT):
                    vps = pspool.tile([128, 128], F32, tag="mmps", name="vps")
                    for c in range(4):
                        nc.tensor.matmul(
                            vps,
                            lhsT=zt[:, c, t * 128:(t + 1) * 128],
                            rhs=wv_sb[:, c, :],
                            start=(c == 0),
                            stop=(c == 3),
                        )
                    nc.vector.tensor_copy(
                        v_sb[:, :, t, 1:65],
                        vps.rearrange("p (s d) -> p s d", s=2),
                    )

                # ---- NF stats from bf16 natural-layout Q ------------------
                # xc starts as Q = z @ Wq + bq, then centered in place.
                xc = const.tile([128, NT, 2, DH], F32, tag="xc")
                xc_flat = xc.rearrange("p t s d -> p t (s d)")
                for t in range(NT):
                    psq = pspool.tile([128, 128], F32, tag="mmps", name="psq")
                    for c in range(4):
                        nc.tensor.matmul(
                            psq,
                            lhsT=zt_bf[:, c, t * 128:(t + 1) * 128],
                            rhs=wq_bf[:, c, :],
                            start=(c == 0),
                            stop=False,
                        )
                    nc.tensor.matmul(psq, lhsT=onesq_bf, rhs=bqr_bf,
                                     start=False, stop=True)
                    nc.vector.tensor_copy(xc_flat[:, t, :], psq)

                musum = const.tile([128, NT, 2], F32, tag="musum")
                nc.vector.tensor_reduce(musum, xc, axis=mybir.AxisListType.X,
                                        op=ALU.add)
                mu = const.tile([128, NT, 2], F32, tag="mu")
                nc.vector.tensor_single_scalar(mu, musum, 1.0 / DH, ALU.mult)
                mu_ap = mu[:, :, :]
                mu_b = bass.AP(tensor=mu_ap.tensor, offset=mu_ap.offset,
                               ap=list(mu_ap.ap) + [[0, DH]])
                nc.vector.tensor_tensor(xc, xc, mu_b, ALU.subtract)

                s2 = const.tile([128, NT, 2], F32, tag="s2")
                xc2 = const.tile([128, NT, 2, DH], F32, tag="xc2")
                nc.vector.tensor_mul(xc2, xc, xc)
                nc.vector.tensor_reduce(s2, xc2, axis=mybir.AxisListType.X,
                                        op=ALU.add)
                s3 = const.tile([128, NT, 2], F32, tag="s3")
                nc.vector.tensor_reduce(s3, xc, axis=mybir.AxisListType.X,
                                        op=ALU.add, apply_absolute_value=True)

                sig = const.tile([128, NT, 2], F32, tag="sig")
                nc.scalar.activation(sig, s2, AF.Sqrt, scale=1.0 / DH,
                                     bias=eps_b[:, :])
                nc.vector.tensor_single_scalar(sig, sig, EPS, ALU.add)
                rsig = const.tile([128, NT, 2], F32, tag="rsig")
                nc.vector.reciprocal(rsig, sig)
                nf = const.tile([128, NT, 2], F32, tag="nf")
                nc.vector.tensor_mul(nf, s3, rsig)
                gt = const.tile([128, NT, 2], F32R, tag="gt")
                nc.scalar.activation(gt, nf, AF.Sigmoid)

                # c/g into query-ordered rows; two-hop SBUF->DRAM->SBUF (the
                # direct partition-crossing reshape DMA exceeds the 3-dim
                # AP-balancing limit).
                for s in (0, 1):
                    eng = nc.sync
                    gdram = dramp.tile([128, NT], F32R, tag=f"gdram{s}",
                                       name=f"gdram{s}")
                    eng.dma_start(out=gdram, in_=gt[:, :, s])
                    eng.dma_start(out=qtaug[s][64:65, :],
                                  in_=gdram.rearrange("p t -> t p"))
                    eng.dma_start(out=grow[s][0:1, :],
                                  in_=gdram.rearrange("p t -> t p"))

                # ---- per head: Q proj + augmentation, -k2 row, attention --
                for s in (0, 1):
                    for qs in range(NQS):
                        sl = slice(qs * 512, (qs + 1) * 512)
                        qps = pspool.tile([64, 512], F32, tag="mmps", name="qps")
                        for c in range(4):
                            nc.tensor.matmul(
                                qps,
                                lhsT=wq_sb[:, c, s * 64:(s + 1) * 64],
                                rhs=zt[:, c, sl],
                                start=(c == 0),
                                stop=(c == 3),
                            )
                        cb = cbp.tile([64, 512], F32, tag="cb")
                        nc.gpsimd.partition_broadcast(cb, grow[s][0:1, sl].bitcast(F32))
                        nc.vector.tensor_scalar(cb, cb, 2.0, 0.125,
                                                ALU.mult, ALU.add)
                        nc.vector.scalar_tensor_tensor(
                            out=qtaug[s][0:64, sl],
                            in0=qps,
                            scalar=bq_sb[:, s:s + 1],
                            in1=cb,
                            op0=ALU.add,
                            op1=ALU.mult,
                        )

                    # -k2 row: square KT (incl. bias) on DVE, col-sum via
                    # (-1)-vector matmul
                    kt2 = small.tile([64, N], F32R, tag="kt2", bufs=1)
                    nc.vector.tensor_mul(kt2, ktaug[s][0:64, :], ktaug[s][0:64, :])
                    nk2row = small.tile([1, N], F32R, tag="nk2row", bufs=1)
                    for qs in range(NQS):
                        sl = slice(qs * 512, (qs + 1) * 512)
                        k2ps = pspool.tile([1, 512], F32, tag="mmps", name="k2ps")
                        nc.tensor.matmul(k2ps, lhsT=nones64, rhs=kt2[:, sl],
                                         start=True, stop=True)
                        nc.vector.tensor_copy(nk2row[0:1, sl], k2ps)
                    nc.sync.dma_start(out=ktaug[s][64:65, :], in_=nk2row[0:1, :])

                    # ---- attention main loop: scores^T -> exp -> PV -------
                    for g in range(NQG):
                        gsl = slice(g * QW, (g + 1) * QW)
                        ot = pspool.tile([65, QW], F32, tag="ot", bufs=1,
                                         name="ot")

                        def exp_pv(kt, st):
                            at = atp.tile([128, QW], F32R, tag="at", name="at")
                            nc.scalar.activation(at, st, AF.Exp)
                            for h in range(2):
                                hs = slice(h * 512, (h + 1) * 512)
                                nc.tensor.matmul(
                                    ot[:, hs],
                                    lhsT=v_sb[:, s, kt, :],
                                    rhs=at[:, hs],
                                    start=(kt == 0),
                                    stop=(kt == KC - 1),
                                )

                        prev_kt = prev_st = None
                        for kt in range(KC):
                            st = pspool.tile([128, QW], F32, tag="st", bufs=2,
                                             name="st")
                            for h in range(2):
                                nc.tensor.matmul(
                                    st[:, h * 512:(h + 1) * 512],
                                    lhsT=ktaug[s][:, kt * 128:(kt + 1) * 128],
                                    rhs=qtaug[s][:, g * QW + h * 512:
                                                 g * QW + (h + 1) * 512],
                                    start=True,
                                    stop=True,
                                )
                            if prev_st is not None:
                                exp_pv(prev_kt, prev_st)
                            prev_kt, prev_st = kt, st
                        exp_pv(prev_kt, prev_st)

                        rsb = small.tile([1, QW], F32, tag="rsb")
                        nc.vector.reciprocal(rsb, ot[0:1, :])
                        rb = cbp.tile([65, QW], F32, tag="rb", bufs=1)
                        nc.gpsimd.partition_broadcast(rb, rsb)
                        nc.vector.tensor_mul(otn[s][:, gsl], ot, rb)

                # ---- out-projection --------------------------------------
                for tp in range(NT // 2):
                    ob = osbp.tile([128, 2, 512], F32, tag="ob")
                    for u in (0, 1):
                        t = tp * 2 + u
                        op = pspool.tile([128, 512], F32, tag="mmps", name="op")
                        nc.tensor.matmul(op,
                                         lhsT=otn[0][:, t * 128:(t + 1) * 128],
                                         rhs=wo_sb[0], start=True, stop=False)
                        nc.tensor.matmul(op,
                                         lhsT=otn[1][:, t * 128:(t + 1) * 128],
                                         rhs=wo_sb[1], start=False, stop=True)
                        nc.vector.tensor_copy(ob[:, u, :], op)
                    eng = nc.sync
                    eng.dma_start(
                        out=out.rearrange("(a p) m -> p a m", p=128)[:, tp * 2:tp * 2 + 2, :],
                        in_=ob)


_CACHE = {}


def _get_nc():
    if "nc" not in _CACHE:
        _CACHE["nc"] = build_nc()
    return _CACHE["nc"]


def make_in_maps(z, Wq, bq, Wk, bk, Wv, Wo):
    in_maps = []
    for core in range(8):
        b = core // 4
        h0 = (core % 4) * 2
        cols = slice(h0 * 64, h0 * 64 + 128)
        woaug = np.zeros((2, 65, DIM), np.float32)
        for s in (0, 1):
            woaug[s, 1:65, :] = Wo[(h0 + s) * 64:(h0 + s + 1) * 64, :]
        in_maps.append({
            "zT": np.ascontiguousarray(z[b].T),
            "wq": np.ascontiguousarray(Wq[:, cols]),
            "wk": np.ascontiguousarray(Wk[:, cols]),
            "wv": np.ascontiguousarray(Wv[:, cols]),
            "bq2": np.ascontiguousarray(bq[cols].reshape(2, 64).T),
            "bk2": np.ascontiguousarray(bk[cols].reshape(2, 64).T),
            "bqr": np.ascontiguousarray(bq[cols].reshape(1, 128)),
            "woaug": woaug,
        })
    return in_maps


def kernel(z, Wq, bq, Wk, bk, Wv, bv, Wo, bo, **run_kwargs):
    z = np.asarray(z, np.float32)
    Wq = np.asarray(Wq, np.float32)
    bq = np.asarray(bq, np.float32)
    Wk = np.asarray(Wk, np.float32)
    bk = np.asarray(bk, np.float32)
    Wv = np.asarray(Wv, np.float32)
    bv = np.asarray(bv, np.float32)
    Wo = np.asarray(Wo, np.float32)
    bo = np.asarray(bo, np.float32)

    in_maps = make_in_maps(z, Wq, bq, Wk, bk, Wv, Wo)
    res = run_bass_kernel_spmd(_get_nc(), in_maps, list(range(8)), **run_kwargs)

    # A's rows sum to 1 exactly, so the V-bias contribution collapses into a
    # constant row added once per batch: bo_eff = bo + bv @ Wo.
    bo_eff = bo + bv @ Wo
    out = np.zeros((2, N, DIM), np.float32)
    for core in range(8):
        out[core // 4] += res.results[core]["out"]
    out += bo_eff[None, None, :]
    if run_kwargs:
        kernel.last_result = res
    return out


# revision 23
# speedup vs baseline: 1.6505x; 1.6505x over previous
"""AttractorGuidedNeuroFocalAttention — Trainium2 Bass kernel (8 cores).

Math: reference logits = qk/sqrt(64) - omega, where
  omega = (q2_i + k2_j - 2*qk_ij) * g_i,   g_i = sigmoid(nf_i)
  => logits_ij = (1/8 + 2*g_i)*qk_ij - g_i*k2_j - g_i*q2_i
The "- g_i*q2_i" term is constant along the softmax axis (j) and cancels.
So with c_i = 1/8 + 2*g_i the whole pre-softmax computation folds into one
65-dim augmented matmul:  logits'_ij = [c_i*Q_i, g_i] . [K_j, -k2_j].

Sharding: 16 (batch, head) pairs over 8 cores -> 2 heads per core.
Each core computes scores transposed (partition = keys) so softmax
normalization comes out of the PV matmul for free via an all-ones column
prepended to V (row 0 of the PV output = sum_j exp(logits)_ij).  The final
out-projection partials [2048, 512] are summed on the host per batch.
"""

import sys

for _p in ("/opt/trn_rl_repo", "/root/.axon_site/_ro/trn_rl_repo"):
    if _p not in sys.path:
        sys.path.insert(0, _p)

import numpy as np

import concourse.bacc as bacc
import concourse.bass as bass
import concourse.tile as tile
from concourse import mybir
from concourse.bass_utils import run_bass_kernel_spmd

F32 = mybir.dt.float32
F32R = mybir.dt.float32r
BF16 = mybir.dt.bfloat16
AF = mybir.ActivationFunctionType
ALU = mybir.AluOpType

N = 2048      # sequence length
DIM = 512     # model dim
DH = 64       # head dim
NT = 16       # 128-token tiles
NQS = 4       # 512-query slices (projections)
NQG = 2       # 1024-query groups (attention inner loop)
QW = 1024     # attention query-group width
KC = 16       # 128-key chunks
EPS = 1e-5


def build_nc(niter=1):
    nc = bacc.Bacc(None, target_bir_lowering=False)
    _build_body(nc, niter)
    nc.finalize()
    return nc


def _build_body(nc, niter=1):
    zT = nc.declare_dram_parameter("zT", [DIM, N], F32R, isOutput=False)
    wq = nc.declare_dram_parameter("wq", [DIM, 128], F32R, isOutput=False)
    wk = nc.declare_dram_parameter("wk", [DIM, 128], F32R, isOutput=False)
    wv = nc.declare_dram_parameter("wv", [DIM, 128], F32R, isOutput=False)
    bq2 = nc.declare_dram_parameter("bq2", [64, 2], F32, isOutput=False)
    bk2 = nc.declare_dram_parameter("bk2", [64, 2], F32, isOutput=False)
    bqr = nc.declare_dram_parameter("bqr", [1, 128], F32, isOutput=False)
    woaug = nc.declare_dram_parameter("woaug", [2, 65, DIM], F32R, isOutput=False)
    out = nc.declare_dram_parameter("out", [N, DIM], F32, isOutput=True)

    with tile.TileContext(nc) as tc:
        with (
            tc.tile_pool(name="const", bufs=1) as const,
            tc.tile_pool(name="small", bufs=2) as small,
            tc.tile_pool(name="atp", bufs=2) as atp,
            tc.tile_pool(name="cbp", bufs=2) as cbp,
            tc.tile_pool(name="osbp", bufs=2) as osbp,
            tc.tile_pool(name="dramp", bufs=1, space="DRAM") as dramp,
            tc.tile_pool(name="pspool", bufs=2, space="PSUM") as pspool,
        ):
            # ---------------- Phase A: loads + casts + constants ----------
            zt = const.tile([128, 4, N], F32R, tag="zt")
            zt_bf = const.tile([128, 4, N], BF16, tag="zt_bf")
            zTv = zT.rearrange("(c p) n -> p c n", p=128)
            nc.sync.dma_start(out=zt[:, 0:2, :], in_=zTv[:, 0:2, :])
            nc.gpsimd.dma_start(out=zt[:, 2:4, :], in_=zTv[:, 2:4, :])
            for c in range(4):
                nc.gpsimd.tensor_copy(zt_bf[:, c, :], zt[:, c, :])

            wq_sb = const.tile([128, 4, 128], F32R, tag="wq")
            wk_sb = const.tile([128, 4, 128], F32R, tag="wk")
            wv_sb = const.tile([128, 4, 128], F32R, tag="wv")
            nc.gpsimd.dma_start(out=wq_sb, in_=wq.rearrange("(c p) m -> p c m", p=128))
            nc.gpsimd.dma_start(out=wk_sb, in_=wk.rearrange("(c p) m -> p c m", p=128))
            nc.gpsimd.dma_start(out=wv_sb, in_=wv.rearrange("(c p) m -> p c m", p=128))
            wq_bf = const.tile([128, 4, 128], BF16, tag="wq_bf")
            nc.gpsimd.tensor_copy(wq_bf, wq_sb)

            bq_sb = const.tile([64, 2], F32, tag="bq")
            bk_sb = const.tile([64, 2], F32, tag="bk")
            nc.gpsimd.dma_start(out=bq_sb, in_=bq2[:, :])
            nc.gpsimd.dma_start(out=bk_sb, in_=bk2[:, :])
            bqr_sb = const.tile([1, 128], F32, tag="bqr")
            nc.gpsimd.dma_start(out=bqr_sb, in_=bqr[:, :])
            bqr_bf = const.tile([1, 128], BF16, tag="bqr_bf")
            nc.vector.tensor_copy(bqr_bf, bqr_sb)

            wo_sb = [const.tile([65, DIM], F32R, tag=f"wo{s}", name=f"wo{s}")
                     for s in (0, 1)]
            for s in (0, 1):
                nc.gpsimd.dma_start(out=wo_sb[s], in_=woaug[s, :, :])

            nones64_f = const.tile([64, 1], F32, tag="nones64_f")
            nc.vector.memset(nones64_f, -1.0)
            nones64 = const.tile([64, 1], F32R, tag="nones64")
            nc.vector.tensor_copy(nones64, nones64_f)
            ones_f = const.tile([128, 1], F32, tag="ones_f")
            nc.vector.memset(ones_f, 1.0)
            onesq_bf = const.tile([1, 128], BF16, tag="onesq_bf")
            nc.vector.memset(onesq_bf, 1.0)
            eps_b = const.tile([128, 1], F32, tag="eps_b")
            nc.vector.memset(eps_b, EPS)

            for _it in range(niter):
                qtaug = [const.tile([65, N], F32R, tag=f"qtaug{s}",
                                    name=f"qtaug{s}") for s in (0, 1)]
                ktaug = [const.tile([65, N], F32R, tag=f"ktaug{s}",
                                    name=f"ktaug{s}") for s in (0, 1)]
                otn = [const.tile([65, N], F32R, tag=f"otn{s}",
                                  name=f"otn{s}") for s in (0, 1)]
                grow = [const.tile([1, N], F32R, tag=f"grow{s}",
                                   name=f"grow{s}") for s in (0, 1)]

                # ---- K projection (starts as soon as zt slices land) ------
                for s in (0, 1):
                    for qs in range(NQS):
                        sl = slice(qs * 512, (qs + 1) * 512)
                        kps = pspool.tile([64, 512], F32, tag="mmps", name="kps")
                        for c in range(4):
                            nc.tensor.matmul(
                                kps,
                                lhsT=wk_sb[:, c, s * 64:(s + 1) * 64],
                                rhs=zt[:, c, sl],
                                start=(c == 0),
                                stop=(c == 3),
                            )
                        nc.vector.tensor_scalar_add(ktaug[s][0:64, sl], kps,
                                                    bk_sb[:, s:s + 1])

                # ---- V projection (f32r, ones col prepended) --------------
                v_sb = const.tile([128, 2, NT, 65], F32R, tag="v")
                ones_ap = ones_f[:, :]
                ones_bcast = bass.AP(tensor=ones_ap.tensor, offset=ones_ap.offset,
                                     ap=[list(ones_ap.ap[0]), [0, 2], [0, NT]])
                nc.vector.tensor_copy(v_sb[:, :, :, 0], ones_bcast)
                for t in range(NT):
                    vps = pspool.tile([128, 128], F32, tag="mmps", name="vps")
                    for c in range(4):
                        nc.tensor.matmul(
                            vps,
                            lhsT=zt[:, c, t * 128:(t + 1) * 128],
                            rhs=wv_sb[:, c, :],
                            start=(c == 0),
                            stop=(c == 3),
                        )
                    nc.vector.tensor_copy(
                        v_sb[:, :, t, 1:65],
                        vps.rearrange("p (s d) -> p s d", s=2),
                    )

                # ---- NF stats from bf16 natural-layout Q ------------------
                # xc starts as Q = z @ Wq + bq, then centered in place.
                xc = const.tile([128, NT, 2, DH], F32, tag="xc")
                xc_flat = xc.rearrange("p t s d -> p t (s d)")
                for t in range(NT):
                    psq = pspool.tile([128, 128], F32, tag="mmps", name="psq")
                    for c in range(4):
                        nc.tensor.matmul(
                            psq,
                            lhsT=zt_bf[:, c, t * 128:(t + 1) * 128],
                            rhs=wq_bf[:, c, :],
                            start=(c == 0),
                            stop=False,
                        )
                    nc.tensor.matmul(psq, lhsT=onesq_bf, rhs=bqr_bf,
                                     start=False, stop=True)
                    nc.vector.tensor_copy(xc_flat[:, t, :], psq)

                musum = const.tile([128, NT, 2], F32, tag="musum")
                nc.vector.tensor_reduce(musum, xc, axis=mybir.AxisListType.X,
                                        op=ALU.add)
                mu = const.tile([128, NT, 2], F32, tag="mu")
                nc.vector.tensor_single_scalar(mu, musum, 1.0 / DH, ALU.mult)
                mu_ap = mu[:, :, :]
                mu_b = bass.AP(tensor=mu_ap.tensor, offset=mu_ap.offset,
                               ap=list(mu_ap.ap) + [[0, DH]])
                nc.vector.tensor_tensor(xc, xc, mu_b, ALU.subtract)

                s2 = const.tile([128, NT, 2], F32, tag="s2")
                xc2 = const.tile([128, NT, 2, DH], F32, tag="xc2")
                nc.vector.tensor_mul(xc2, xc, xc)
                nc.vector.tensor_reduce(s2, xc2, axis=mybir.AxisListType.X,
                                        op=ALU.add)
                s3 = const.tile([128, NT, 2], F32, tag="s3")
                nc.vector.tensor_reduce(s3, xc, axis=mybir.AxisListType.X,
                                        op=ALU.add, apply_absolute_value=True)

                sig = const.tile([128, NT, 2], F32, tag="sig")
                nc.scalar.activation(sig, s2, AF.Sqrt, scale=1.0 / DH,
                                     bias=eps_b[:, :])
                nc.vector.tensor_single_scalar(sig, sig, EPS, ALU.add)
                rsig = const.tile([128, NT, 2], F32, tag="rsig")
                nc.vector.reciprocal(rsig, sig)
                nf = const.tile([128, NT, 2], F32, tag="nf")
                nc.vector.tensor_mul(nf, s3, rsig)
                gt = const.tile([128, NT, 2], F32R, tag="gt")
                nc.scalar.activation(gt, nf, AF.Sigmoid)

                # c/g into query-ordered rows; two-hop SBUF->DRAM->SBUF (the
                # direct partition-crossing reshape DMA exceeds the 3-dim
                # AP-balancing limit).
                for s in (0, 1):
                    eng = nc.sync if s == 0 else nc.gpsimd
                    gdram = dramp.tile([128, NT], F32R, tag=f"gdram{s}",
                                       name=f"gdram{s}")
                    eng.dma_start(out=gdram, in_=gt[:, :, s])
                    eng.dma_start(out=qtaug[s][64:65, :],
                                  in_=gdram.rearrange("p t -> t p"))
                    eng.dma_start(out=grow[s][0:1, :],
                                  in_=gdram.rearrange("p t -> t p"))

                # ---- per head: Q proj + augmentation, -k2 row, attention --
                for s in (0, 1):
                    for qs in range(NQS):
                        sl = slice(qs * 512, (qs + 1) * 512)
                        qps = pspool.tile([64, 512], F32, tag="mmps", name="qps")
                        for c in range(4):
                            nc.tensor.matmul(
                                qps,
                                lhsT=wq_sb[:, c, s * 64:(s + 1) * 64],
                                rhs=zt[:, c, sl],
                                start=(c == 0),
                                stop=(c == 3),
                            )
                        cb = cbp.tile([64, 512], F32, tag="cb")
                        nc.gpsimd.partition_broadcast(cb, grow[s][0:1, sl].bitcast(F32))
                        nc.vector.tensor_scalar(cb, cb, 2.0, 0.125,
                                                ALU.mult, ALU.add)
                        nc.vector.scalar_tensor_tensor(
                            out=qtaug[s][0:64, sl],
                            in0=qps,
                            scalar=bq_sb[:, s:s + 1],
                            in1=cb,
                            op0=ALU.add,
                            op1=ALU.mult,
                        )

                    # -k2 row: square KT (incl. bias) on DVE, col-sum via
                    # (-1)-vector matmul
                    kt2 = small.tile([64, N], F32R, tag="kt2", bufs=1)
                    nc.vector.tensor_mul(kt2, ktaug[s][0:64, :], ktaug[s][0:64, :])
                    nk2row = small.tile([1, N], F32R, tag="nk2row", bufs=1)
                    for qs in range(NQS):
                        sl = slice(qs * 512, (qs + 1) * 512)
                        k2ps = pspool.tile([1, 512], F32, tag="mmps", name="k2ps")
                        nc.tensor.matmul(k2ps, lhsT=nones64, rhs=kt2[:, sl],
                                         start=True, stop=True)
                        nc.vector.tensor_copy(nk2row[0:1, sl], k2ps)
                    (nc.sync if s == 0 else nc.gpsimd).dma_start(out=ktaug[s][64:65, :], in_=nk2row[0:1, :])

                    # ---- attention main loop: scores^T -> exp -> PV -------
                    for g in range(NQG):
                        gsl = slice(g * QW, (g + 1) * QW)
                        ot = pspool.tile([65, QW], F32, tag="ot", bufs=1,
                                         name="ot")

                        def exp_pv(kt, st):
                            at = atp.tile([128, QW], F32R, tag="at", name="at")
                            nc.scalar.activation(at, st, AF.Exp)
                            for h in range(2):
                                hs = slice(h * 512, (h + 1) * 512)
                                nc.tensor.matmul(
                                    ot[:, hs],
                                    lhsT=v_sb[:, s, kt, :],
                                    rhs=at[:, hs],
                                    start=(kt == 0),
                                    stop=(kt == KC - 1),
                                )

                        prev_kt = prev_st = None
                        for kt in range(KC):
                            st = pspool.tile([128, QW], F32, tag="st", bufs=2,
                                             name="st")
                            for h in range(2):
                                nc.tensor.matmul(
                                    st[:, h * 512:(h + 1) * 512],
                                    lhsT=ktaug[s][:, kt * 128:(kt + 1) * 128],
                                    rhs=qtaug[s][:, g * QW + h * 512:
                                                 g * QW + (h + 1) * 512],
                                    start=True,
                                    stop=True,
                                )
                            if prev_st is not None:
                                exp_pv(prev_kt, prev_st)
                            prev_kt, prev_st = kt, st
                        exp_pv(prev_kt, prev_st)

                        rsb = small.tile([1, QW], F32, tag="rsb")
                        nc.vector.reciprocal(rsb, ot[0:1, :])
                        rb = cbp.tile([65, QW], F32, tag="rb", bufs=1)
                        nc.gpsimd.partition_broadcast(rb, rsb)
                        nc.vector.tensor_mul(otn[s][:, gsl], ot, rb)

                # ---- out-projection --------------------------------------
                for tp in range(NT // 2):
                    ob = osbp.tile([128, 2, 512], F32, tag="ob")
                    for u in (0, 1):
                        t = tp * 2 + u
                        op = pspool.tile([128, 512], F32, tag="mmps", name="op")
                        nc.tensor.matmul(op,
                                         lhsT=otn[0][:, t * 128:(t + 1) * 128],
                                         rhs=wo_sb[0], start=True, stop=False)
                        nc.tensor.matmul(op,
                                         lhsT=otn[1][:, t * 128:(t + 1) * 128],
                                         rhs=wo_sb[1], start=False, stop=True)
                        nc.vector.tensor_copy(ob[:, u, :], op)
                    eng = nc.sync if tp % 2 == 0 else nc.gpsimd
                    eng.dma_start(
                        out=out.rearrange("(a p) m -> p a m", p=128)[:, tp * 2:tp * 2 + 2, :],
                        in_=ob)


_CACHE = {}


def _get_nc():
    if "nc" not in _CACHE:
        _CACHE["nc"] = build_nc()
    return _CACHE["nc"]


def make_in_maps(z, Wq, bq, Wk, bk, Wv, Wo):
    in_maps = []
    for core in range(8):
        b = core // 4
        h0 = (core % 4) * 2
        cols = slice(h0 * 64, h0 * 64 + 128)
        woaug = np.zeros((2, 65, DIM), np.float32)
        for s in (0, 1):
            woaug[s, 1:65, :] = Wo[(h0 + s) * 64:(h0 + s + 1) * 64, :]
        in_maps.append({
            "zT": np.ascontiguousarray(z[b].T),
            "wq": np.ascontiguousarray(Wq[:, cols]),
            "wk": np.ascontiguousarray(Wk[:, cols]),
            "wv": np.ascontiguousarray(Wv[:, cols]),
            "bq2": np.ascontiguousarray(bq[cols].reshape(2, 64).T),
            "bk2": np.ascontiguousarray(bk[cols].reshape(2, 64).T),
            "bqr": np.ascontiguousarray(bq[cols].reshape(1, 128)),
            "woaug": woaug,
        })
    return in_maps


def kernel(z, Wq, bq, Wk, bk, Wv, bv, Wo, bo, **run_kwargs):
    z = np.asarray(z, np.float32)
    Wq = np.asarray(Wq, np.float32)
    bq = np.asarray(bq, np.float32)
    Wk = np.asarray(Wk, np.float32)
    bk = np.asarray(bk, np.float32)
    Wv = np.asarray(Wv, np.float32)
    bv = np.asarray(bv, np.float32)
    Wo = np.asarray(Wo, np.float32)
    bo = np.asarray(bo, np.float32)

    in_maps = make_in_maps(z, Wq, bq, Wk, bk, Wv, Wo)
    res = run_bass_kernel_spmd(_get_nc(), in_maps, list(range(8)), **run_kwargs)

    # A's rows sum to 1 exactly, so the V-bias contribution collapses into a
    # constant row added once per batch: bo_eff = bo + bv @ Wo.
    bo_eff = bo + bv @ Wo
    out = np.zeros((2, N, DIM), np.float32)
    for core in range(8):
        out[core // 4] += res.results[core]["out"]
    out += bo_eff[None, None, :]
    if run_kwargs:
        kernel.last_result = res
    return out
